# revision 49
# baseline (speedup 1.0000x reference)
"""Multi-head attention (B=2, S=2048, D=1024, H=16) on 8 TRN2 NeuronCores.

Sharding: batch x head-group. Core c handles batch b = c // 4 and heads
[4*(c%4), 4*(c%4)+4). Each core projects Q/K/V for its 4 heads (column-split
wq/wk/wv), runs causal attention per head, and computes its partial of the
output projection (row-split wo). Host sums the 4 partials per batch (the
"all-reduce") and adds wo_b.

Device-side design (v2 — interleaved phases, rebalanced engines):
  - Host supplies q/k/v transposed (xT = x[b].T, [D, S]) so the projection
    contraction dim (D) lands on SBUF partitions with no on-device transpose.
  - Q,K are produced transposed (QT[dout, s]); scores are computed in S^T
    layout [keys, queries]; softmax uses no max-subtraction (scores/8 lie in
    [-3, 3] for randn inputs; exp cannot overflow).
  - Width-65 V strips [64 dims | ones]: the A@V matmul emits both the context
    rows (partitions 0..63) and the softmax denominator (partition 64) per
    head; denominators are inverted on DVE (reciprocal) and broadcast to 128
    partitions with one tiny K=2 matmul — no DRAM round-trips, no Exp<->Ln
    activation-table swaps on the scalar engine.
  - Causal masking by construction: per (query-chunk, key-block), only the
    live query range [128*al, 512) is computed (scores, exp, A@V); just the
    128-wide diagonal transition band needs a triangular mask multiply,
    which runs on the otherwise idle GpSimd engine.
  - Projection (phase A) and attention (phase B) instruction issue is
    interleaved so the tensor engine's projection work overlaps the scalar
    engine's exp work instead of serializing.
  - All big DMAs are split across queues; output partials are bf16.
"""
import math
import os
import numpy as np
from contextlib import ExitStack

B, S, D, H = 2, 2048, 1024, 16
DK = D // H               # 64
NCORES = 8
HPC = H // (NCORES // B)  # heads per core = 4
DHC = HPC * DK            # per-core head dims = 256
P = 128
SCW = 512
NSC = S // SCW            # 4 s-chunks (= query chunks)
NKC = D // P              # 8 contraction chunks
NQB = S // P              # 16 key blocks

_compiled = {}


def _build(mode: str):
    """mode: 'causal' (live-range restricted, const band mask),
             'dense'  (no masking at all),
             'general' (full SxS additive bias streamed from DRAM)."""
    import concourse.bacc as bacc
    import concourse.mybir as mybir
    import concourse.tile as tile

    f32 = mybir.dt.float32
    bf16 = mybir.dt.bfloat16
    fp16 = mybir.dt.float16
    AF = mybir.ActivationFunctionType
    nc = bacc.Bacc("TRN2", target_bir_lowering=False, debug=False,
                   num_devices=NCORES)

    qt = nc.dram_tensor("qt", (NSC, P, NKC, SCW), bf16, kind="ExternalInput").ap()
    kt = nc.dram_tensor("kt", (NSC, P, NKC, SCW), bf16, kind="ExternalInput").ap()
    vt = nc.dram_tensor("vt", (NSC, P, NKC, SCW), bf16, kind="ExternalInput").ap()
    wq = nc.dram_tensor("wq", (P, NKC, DHC), bf16, kind="ExternalInput").ap()
    wk = nc.dram_tensor("wk", (P, NKC, DHC), bf16, kind="ExternalInput").ap()
    wv = nc.dram_tensor("wv", (P, NKC, DHC), bf16, kind="ExternalInput").ap()
    wo = nc.dram_tensor("wo", (P, DHC // P, D), bf16, kind="ExternalInput").ap()
    # aux: [0] = 1.0, [1:257] = wq bias for this core's 256 head-dims
    aux = nc.dram_tensor("aux", (1, 257), bf16, kind="ExternalInput").ap()
    if mode == "causal":
        maskb = nc.dram_tensor("maskb", (P, P), bf16, kind="ExternalInput").ap()
    elif mode == "general":
        maskt = nc.dram_tensor("maskt", (S, S), f32, kind="ExternalInput").ap()
    outT = nc.dram_tensor("outT", (D, S), bf16, kind="ExternalOutput").ap()

    with tile.TileContext(nc) as tc, ExitStack() as ctx:
        consts = ctx.enter_context(tc.tile_pool(name="consts", bufs=1))
        stream = ctx.enter_context(tc.tile_pool(name="stream", bufs=3))
        espool = ctx.enter_context(tc.tile_pool(name="es", bufs=8))
        epool = ctx.enter_context(tc.tile_pool(name="ep", bufs=4))
        opool = ctx.enter_context(tc.tile_pool(name="op", bufs=4))
        rpool = ctx.enter_context(tc.tile_pool(name="rp", bufs=2))
        # PSUM: acc 2x1 bank + av 2x1 + sc 2x2 = 8 banks total
        acc_ps = ctx.enter_context(tc.tile_pool(name="accps", bufs=2, space="PSUM"))
        av_ps = ctx.enter_context(tc.tile_pool(name="avps", bufs=2, space="PSUM"))
        sc_ps = ctx.enter_context(tc.tile_pool(name="scps", bufs=2, space="PSUM"))

        # ---- resident tensors ----
        wq_sb = consts.tile([P, NKC, DHC], bf16, tag="wq")
        wk_sb = consts.tile([P, NKC, DHC], bf16, tag="wk")
        wv_sb = consts.tile([P, NKC, DHC], bf16, tag="wv")
        wo_sb = consts.tile([P, DHC // P, D], bf16, tag="wo")
        aux_sb = consts.tile([1, 257], bf16, tag="aux")
        ones64_sb = consts.tile([P, 64], bf16, tag="ones64")
        qb_sb = consts.tile([P, 2], f32, tag="qb")
        QT_sb = consts.tile([P, 2, S], bf16, tag="QT")
        KT_sb = consts.tile([P, 2, S], bf16, tag="KT")
        # V strips: [key-in-block, sb, pair, [h_even 64|1][h_odd 64|1]]
        V_sb = consts.tile([P, NQB, 2, 130], bf16, tag="V")
        ctx_sb = consts.tile([P, 2, S], bf16, tag="ctx")
        if mode == "causal":
            maskb_sb = consts.tile([P, P], bf16, tag="maskb")

        warm_sb = consts.tile([P, SCW], bf16, tag="warm")

        def init_consts():
            # on-device constants: ones columns of the V strips (softmax
            # denominator rows) and the bcmul broadcast ones — replaces
            # thousands of tiny DMA descriptors with 3 memsets
            nc.gpsimd.memset(V_sb[:, :, :, 64:65], 1.0)
            nc.gpsimd.memset(V_sb[:, :, :, 129:130], 1.0)
            nc.vector.memset(ones64_sb[:], 1.0)
            nc.vector.memset(warm_sb[:], 0.0)

        def warmup():
            # ~4us of dummy matmuls while the first inputs stream in: flips
            # the PE HAM clock gate to 8/8 before the real work arrives
            wp = acc_ps.tile([P, 8, 64], f32, tag="acc", name="accwarm")
            for r in range(10):
                nc.tensor.matmul(wp[:, :, :], warm_sb[:, 0:P], warm_sb[:, :],
                                 start=(r == 0), stop=(r == 9))

        def qb_extract():
            # wq bias column [128, 2] = aux[1:257] via K=1 matmuls
            ps = acc_ps.tile([P, 8, 64], f32, tag="acc")
            for c0 in range(2):
                nc.tensor.matmul(ps[:, c0, 0:1],
                                 aux_sb[:, 1 + P * c0:1 + P * (c0 + 1)],
                                 aux_sb[:, 0:1], start=True, stop=True)
            nc.vector.tensor_copy(qb_sb[:, :], ps[:, 0:2, 0:1])

        def dma_x(sc):
            # two issues per tensor (DMA_DIRECT2D costs ~610ns of issuing-
            # engine time regardless of size): first half arrives early for
            # the projection half-groups, without per-kc issue overhead
            tiles = {}
            for name, src in (("q", qt), ("k", kt), ("v", vt)):
                t = stream.tile([P, NKC, SCW], bf16, tag=f"x{name}")
                nc.sync.dma_start(t[:, 0:4, :], src[sc, :, 0:4, :])
                nc.sync.dma_start(t[:, 4:8, :], src[sc, :, 4:8, :])
                tiles[name] = t
            return tiles

        # ---- Phase A groups: projections for s-chunk sc ----
        def a_groups(sc, xt, split=False):
            gs = []
            ssl = slice(sc * SCW, (sc + 1) * SCW)

            def qk(xkey, w_sb, dst, bias, c0):
                # one whole accumulation chain per closure: the acc psum tile
                # must be allocated, written, and evacuated within one group,
                # or interleaved groups sharing the pool wrap it mid-chain
                def g():
                    ps = acc_ps.tile([P, 8, 64], f32, tag="acc")
                    x = xt[xkey]
                    for kc in range(NKC):
                        nc.tensor.matmul(ps[:, :, :],
                                         w_sb[:, kc, c0 * P:(c0 + 1) * P],
                                         x[:, kc, :],
                                         start=(kc == 0),
                                         stop=(kc == NKC - 1))
                    if bias:
                        # q bias fused into the evacuation copy (per-partition
                        # scalar add). (k bias dropped: per-query score shift,
                        # softmax-invariant; v bias added host-side via wo@bv)
                        nc.vector.tensor_scalar_add(dst[:, c0, ssl], ps[:, :, :],
                                                    qb_sb[:, c0:c0 + 1])
                    else:
                        nc.vector.tensor_copy(dst[:, c0, ssl], ps[:, :, :])
                return g

            for c0 in range(2):
                gs.append(qk("q", wq_sb, QT_sb, True, c0))
            for c0 in range(2):
                gs.append(qk("k", wk_sb, KT_sb, False, c0))

            def vproj(j):
                def g():
                    sb = 4 * sc + j
                    ps = acc_ps.tile([P, 8, 64], f32, tag="acc")
                    pv = ps[:, 0:4, :]
                    for kc in range(NKC):
                        nc.tensor.matmul(pv, xt["v"][:, kc, j * P:(j + 1) * P],
                                         wv_sb[:, kc, :],
                                         start=(kc == 0), stop=(kc == NKC - 1))
                    # wv cols are host-permuted [h0,h2,h1,h3] -> 2 strided copies
                    nc.vector.tensor_copy(V_sb[:, sb, :, 0:DK], ps[:, 0:2, :])
                    nc.vector.tensor_copy(V_sb[:, sb, :, 65:65 + DK], ps[:, 2:4, :])
                return g

            vg = [vproj(j) for j in range(SCW // P)]
            if split:
                return gs, vg
            return gs + vg

        # ---- Phase B groups: attention for query chunk qc ----
        if mode == "general":
            mkpool = ctx.enter_context(tc.tile_pool(name="mk", bufs=1))

        def early_groups(qc, pairs_kbs, store):
            # score+exp only, for off-diagonal key blocks of a later query
            # chunk: pulled forward into the previous (PE-bound) segment so
            # the final segment's scalar-engine exp backlog shrinks
            gs = []
            for pair, kb in pairs_kbs:
                def g(pair=pair, kb=kb):
                    sct = sc_ps.tile([P, 2, SCW], f32, tag="sc")
                    for par in range(2):
                        hp = 64 * par
                        nc.tensor.matmul(
                            sct[:, par, :],
                            KT_sb[hp:hp + 64, pair, kb * P:(kb + 1) * P],
                            QT_sb[hp:hp + 64, pair, qc * SCW:(qc + 1) * SCW],
                            start=True, stop=True, tile_position=(hp, 0))
                    es = epool.tile([P, 2, SCW], bf16, tag="es_e",
                                    name=f"ese{pair}_{kb}")
                    nc.scalar.activation(es[:, :, :], sct[:, :, :], AF.Exp,
                                         scale=1.0 / math.sqrt(DK))
                    store[(pair, kb)] = es
                gs.append(g)
            return gs

        def b_groups(qc, pre=None, early_gs=()):
            gs = []
            pre = pre or {}
            qsl = slice(qc * SCW, (qc + 1) * SCW)
            nkb = 4 * (qc + 1) if mode == "causal" else NQB
            mk_tiles = {}
            if mode == "general":
                def mk_load(g_):
                    def g():
                        mt = mkpool.tile([P, 2, 512], f32, tag=f"mk{g_}")
                        nc.sync.dma_start(
                            mt[:], maskt[2 * g_ * P:(2 * g_ + 2) * P, qsl]
                            .rearrange("(u p) q -> p u q", p=P))
                        mk_tiles[g_] = mt
                    return g
                for g_ in range(nkb // 2):
                    gs.append(("mk", mk_load(g_)))

            avs_by_pair = {}
            # denominator rows at partitions {0, 64} (quadrant-aligned bases;
            # rows 1..63 are junk, never read); free dims: [ch, q]
            Rstg = rpool.tile([65, 2, SCW], f32, tag="Rstg")

            es_by_kb = {}

            def lo_of(kb):
                al = kb - 4 * qc
                return P * al if (mode == "causal" and al > 0) else 0

            def score_part(pair, kb):
                lo = lo_of(kb)
                al = kb - 4 * qc
                sct = sc_ps.tile([P, 2, SCW], f32, tag="sc")
                for par in range(2):
                    hp = 64 * par
                    nc.tensor.matmul(sct[:, par, lo:],
                                     KT_sb[hp:hp + 64, pair, kb * P:(kb + 1) * P],
                                     QT_sb[hp:hp + 64, pair, qc * SCW + lo:(qc + 1) * SCW],
                                     start=True, stop=True,
                                     tile_position=(hp, 0))
                if mode == "general":
                    nc.vector.tensor_add(sct[:, 0, :], sct[:, 0, :],
                                         mk_tiles[kb // 2][:, kb % 2, :])
                    nc.vector.tensor_add(sct[:, 1, :], sct[:, 1, :],
                                         mk_tiles[kb // 2][:, kb % 2, :])
                es = espool.tile([P, 2, SCW], bf16, tag="es")
                nc.scalar.activation(es[:, :, lo:], sct[:, :, lo:], AF.Exp,
                                     scale=1.0 / math.sqrt(DK))
                if mode == "causal" and 0 <= al:
                    # triangular band mask on the diagonal 128 columns
                    nc.gpsimd.tensor_mul(es[:, 0, lo:lo + P], es[:, 0, lo:lo + P],
                                         maskb_sb[:, :])
                    nc.gpsimd.tensor_mul(es[:, 1, lo:lo + P], es[:, 1, lo:lo + P],
                                         maskb_sb[:, :])
                es_by_kb[(pair, kb)] = es

            def av_part(pair, kb):
                lo = lo_of(kb)
                es = es_by_kb.pop((pair, kb))
                if kb == 0:
                    avs_by_pair[pair] = [
                        av_ps.tile([P, SCW], f32, tag="av", name=f"av{pair}{par}")
                        for par in range(2)]
                avs = avs_by_pair[pair]
                for par in range(2):
                    nc.tensor.matmul(avs[par][0:65, lo:],
                                     V_sb[:, kb, pair, par * 65:par * 65 + 65],
                                     es[:, par, lo:],
                                     start=(kb == 0), stop=(kb == nkb - 1))

            def attn(pair, kb):
                # software pipeline: issue av two key-blocks behind the
                # scores so the PE never waits on the exp in program order
                def g():
                    if (pair, kb) not in pre:
                        score_part(pair, kb)
                    if kb >= 2:
                        av_part(pair, kb - 2)
                    if kb == nkb - 1:
                        av_part(pair, nkb - 2)
                        av_part(pair, nkb - 1)
                return g

            def evac_den(pair):
                # den rows first, so recip (DVE) runs before the ctx casts
                # and the bc matmul unblocks ~2us earlier at pair ends
                def g():
                    avs = avs_by_pair[pair]
                    for par in range(2):
                        nc.vector.tensor_copy(Rstg[64 * par:64 * par + 1, pair, :],
                                              avs[par][64:65, :])
                return g

            def evac_ctx(pair):
                def g():
                    avs = avs_by_pair[pair]
                    for par in range(2):
                        if qc == NSC - 1 and pair == 1 and par == 0:
                            # endgame: exp is done, scalar engine is free
                            nc.scalar.copy(
                                ctx_sb[0:64, pair, qsl], avs[par][0:64, :])
                        else:
                            nc.vector.tensor_copy(
                                ctx_sb[64 * par:64 * par + 64, pair, qsl],
                                avs[par][0:64, :])
                return g

            Rf = rpool.tile([65, 2, SCW], f32, tag="Rf")
            Rb = rpool.tile([65, 2, SCW], bf16, tag="Rb")

            def recip(pair):
                def g():
                    with nc.allow_low_precision("softmax denom scale in bf16"):
                        nc.vector.reciprocal_approx_fast(Rf[:, pair, :],
                                                         Rstg[:, pair, :])
                        nc.vector.tensor_copy(Rb[:, pair, :], Rf[:, pair, :])
                return g

            def bcmul(pair):
                def g():
                    bc = acc_ps.tile([P, 8, 64], f32, tag="acc")
                    for par in range(2):
                        nc.tensor.matmul(bc[64 * par:64 * par + 64, :, :],
                                         ones64_sb[64 * par:64 * par + 1, :],
                                         Rb[64 * par:64 * par + 1, pair, :],
                                         start=True, stop=True,
                                         tile_position=(64 * par, 64 * par))
                    nc.vector.tensor_mul(ctx_sb[:, pair, qsl], ctx_sb[:, pair, qsl],
                                         bc[:, :, :])
                return g

            es_by_kb.update(pre)
            ei = 0
            for pair in range(2):
                dkb = 6 if qc == NSC - 1 else 2
                for kb in range(nkb):
                    gs.append(("attn", attn(pair, kb)))
                    if pair == 1 and kb == dkb:
                        gs.append(("bc", bcmul(0)))
                    if pair == 1 and kb >= 3 and kb % 2 == 1 and ei < len(early_gs):
                        gs.append(("early", early_gs[ei]))
                        ei += 1
                gs.append(("evac", evac_den(pair)))
                gs.append(("recip", recip(pair)))
                gs.append(("evacc", evac_ctx(pair)))
            gs.append(("bc", bcmul(1)))
            while ei < len(early_gs):
                gs.append(("early", early_gs[ei]))
                ei += 1

            def outproj(nb):
                def g():
                    ps = acc_ps.tile([P, 8, 64], f32, tag="acc")
                    for hc in range(2):
                        nc.tensor.matmul(ps[:, :, :],
                                         wo_sb[:, hc, nb * P:(nb + 1) * P],
                                         ctx_sb[:, hc, qsl],
                                         start=(hc == 0), stop=(hc == 1))
                    if nb % 2 == 0:
                        oth["t"] = opool.tile([P, 2, SCW], bf16, tag="ot",
                                              name=f"ot{qc}_{nb}")
                    ot = oth["t"]
                    if qc == NSC - 1 and nb % 2 == 1:
                        # endgame: split the evacuation casts across engines
                        # (exp is done, the scalar engine is idle)
                        nc.scalar.copy(ot[:, 1, :], ps[:, :, :])
                    else:
                        nc.vector.tensor_copy(ot[:, nb % 2, :], ps[:, :, :])
                    if nb % 2 == 1:
                        # paired DMA: two nb blocks per issue, 1KB lines
                        h0 = qc * SCW
                        dst = outT[(nb - 1) * P:(nb + 1) * P, h0:h0 + SCW]
                        nc.sync.dma_start(dst.rearrange("(j p) c -> p j c", p=P),
                                          ot[:, :, :])
                return g

            oth = {}
            ops = [outproj(nb) for nb in range(D // P)]
            return gs, ops

        def interleave(bs, as_):
            """Merge phase-A half-groups into the tagged phase-B stream at an
            even rate across all slots: the PE stream is strictly in-order, so
            ~1us of projection work after every attention slot papers over the
            score->exp->av dependency stalls."""
            if not as_:
                for _, g in bs:
                    g()
                return
            slots = ("attn", "evac", "recip", "evacc", "bc", "early", "opd")
            nslots = sum(1 for t, _ in bs if t in slots)
            rate = len(as_) / max(1, nslots)
            ai = 0
            acc = 0.0
            for tag, g in bs:
                g()
                if tag in slots:
                    acc += rate
                    while ai < len(as_) and acc >= 1.0:
                        as_[ai]()
                        ai += 1
                        acc -= 1.0
            while ai < len(as_):
                as_[ai]()
                ai += 1

        # ---- issue ----
        xt = {}
        for name, src in (("q", qt), ("k", kt), ("v", vt)):
            t = stream.tile([P, NKC, SCW], bf16, tag=f"x{name}")
            xt[name] = t
        # Head DMA: DMA_DIRECT2D issue costs ~610ns on the issuing engine,
        # so (a) few, large transfers, (b) split the issue load across both
        # HWDGE queues — sync drives the q-side critical path while the
        # scalar engine (idle until the first exp) drives k/v/weights.
        # single queue, criticality-ordered: arrival order then matches
        # compute order (q -> k -> v); only the late-needed maskb/wo go on
        # the scalar queue
        # interleave wq slices with q chunks in the order the first
        # projection chain consumes them (kc ascending), so it streams
        # without stalling on late weights
        nc.sync.dma_start(wq_sb[:, 0:2, :], wq[:, 0:2, :])
        nc.sync.dma_start(aux_sb[:], aux)
        for g in range(4):
            nc.sync.dma_start(xt["q"][:, 2 * g, :], qt[0, :, 2 * g, :])
            nc.sync.dma_start(xt["q"][:, 2 * g + 1, :], qt[0, :, 2 * g + 1, :])
            if g < 3:
                nc.sync.dma_start(wq_sb[:, 2 * g + 2:2 * g + 4, :],
                                  wq[:, 2 * g + 2:2 * g + 4, :])
        nc.sync.dma_start(wk_sb[:, :, :], wk[:, :, :])
        for g in range(4):
            nc.sync.dma_start(xt["k"][:, 2 * g:2 * g + 2, :],
                              kt[0, :, 2 * g:2 * g + 2, :])
        nc.sync.dma_start(wv_sb[:, :, :], wv[:, :, :])
        for g in range(4):
            nc.sync.dma_start(xt["v"][:, 2 * g:2 * g + 2, :],
                              vt[0, :, 2 * g:2 * g + 2, :])
        if mode == "causal":
            nc.scalar.dma_start(maskb_sb[:], maskb)
        nc.scalar.dma_start(wo_sb[:, :, :], wo[:, :, :])
        init_consts()
        warmup()
        qb_extract()
        for g in a_groups(0, xt):
            g()
        vg0 = []

        def inject_ops(bs, ops):
            # spread deferred out-proj groups after the 3rd..10th attn group
            merged = []
            k = 0
            oi = 0
            for tag, g in bs:
                merged.append((tag, g))
                if tag == "attn":
                    k += 1
                    if k >= 3 and oi < len(ops):
                        merged.append(("opd", ops[oi]))
                        oi += 1
            while oi < len(ops):
                merged.append(("opd", ops[oi]))
                oi += 1
            return merged

        pend = []
        pre3 = {}
        carry = vg0
        for sc in range(1, NSC):
            xt = dma_x(sc)
            egs = ()
            if mode == "causal" and sc == NSC - 1:
                egs = early_groups(NSC - 1, [(0, kb) for kb in range(4)], pre3)
            bs, ops = b_groups(sc - 1, early_gs=egs)
            interleave(inject_ops(bs, pend), carry + a_groups(sc, xt))
            carry = []
            pend = ops
        bs, ops = b_groups(NSC - 1, pre=pre3)
        for _, g in inject_ops(bs, pend):
            g()
        for g in ops:
            g()

    nc.compile()
    return nc


def _get_compiled(mode: str):
    if mode not in _compiled:
        _compiled[mode] = _build(mode)
    return _compiled[mode]


def _detect_mode(mask: np.ndarray) -> str:
    m = np.asarray(mask).reshape(S, S)
    if np.array_equal(m != 0, np.tril(np.ones((S, S), dtype=bool))):
        return "causal"
    if np.all(m != 0):
        return "dense"
    return "general"


def kernel(q, k, v, mask, wq_w, wq_b, wk_w, wk_b, wv_w, wv_b, wo_w, wo_b):
    from concourse import bass_utils

    import ml_dtypes

    q = np.asarray(q, dtype=np.float32)
    k = np.asarray(k, dtype=np.float32)
    v = np.asarray(v, dtype=np.float32)
    mode = _detect_mode(np.asarray(mask))
    nc = _get_compiled(mode)

    def tile_in(x):  # [S, D] -> [sc, p, kc, scw] (x^T pre-tiled for DMA)
        return np.ascontiguousarray(
            x.reshape(S // SCW, SCW, D // P, P).transpose(0, 3, 2, 1)
        ).astype(ml_dtypes.bfloat16)

    def tile_w(w, hs, perm=None):  # [Dout, Din] slice -> W^T tiled [p, kc, DHC]
        ws = w[hs, :]
        if perm is not None:
            ws = ws[perm]
        return np.ascontiguousarray(
            ws.T.reshape(D // P, P, DHC).transpose(1, 0, 2)
        ).astype(ml_dtypes.bfloat16)

    qT = [tile_in(q[b]) for b in range(B)]
    kT = [tile_in(k[b]) for b in range(B)]
    vT = [tile_in(v[b]) for b in range(B)]

    perm = np.r_[0:64, 128:192, 64:128, 192:256]  # head order h0,h2,h1,h3

    if mode == "causal":
        i = np.arange(P)[:, None]
        jb = np.arange(P)[None, :]
        maskb = (jb >= i).astype(ml_dtypes.bfloat16)
    elif mode == "general":
        m = np.asarray(mask).reshape(S, S)
        maskt = np.where(m.T == 0, np.float32(-1.0e9), np.float32(0.0))

    in_maps = []
    for c in range(NCORES):
        b = c // (NCORES // B)
        hg = c % (NCORES // B)
        hs = slice(hg * DHC, (hg + 1) * DHC)
        aux_arr = np.zeros((1, 257), ml_dtypes.bfloat16)
        aux_arr[0, 0] = 1.0
        aux_arr[0, 1:1 + DHC] = wq_b[hs].astype(ml_dtypes.bfloat16)
        m_ = {
            "qt": qT[b], "kt": kT[b], "vt": vT[b],
            "wq": tile_w(wq_w, hs),
            "wk": tile_w(wk_w, hs),
            "wv": tile_w(wv_w, hs, perm),
            "wo": np.ascontiguousarray(
                wo_w[:, hs].T.reshape(2, P, D).transpose(1, 0, 2)
            ).astype(ml_dtypes.bfloat16),
            "aux": aux_arr,
        }
        if mode == "causal":
            m_["maskb"] = maskb
        elif mode == "general":
            m_["maskt"] = maskt
        in_maps.append(m_)

    trace = os.environ.get("KERNEL_TRACE", "") == "1"
    res = bass_utils.run_bass_kernel_spmd(nc, in_maps, core_ids=list(range(NCORES)),
                                          trace=trace)
    if trace:
        kernel.last_exec_time_ns = res.exec_time_ns
        kernel.last_results = res

    # v-projection bias folded here: softmax weights sum to 1, so each
    # head's bv adds a constant; through wo it is wo_w @ wv_b
    out_bias = wo_b + wo_w.astype(np.float64) @ wv_b.astype(np.float64)
    out = np.empty((B, S, D), np.float32)
    for b in range(B):
        acc = res.results[b * (NCORES // B)]["outT"].astype(np.float32)
        for c in range(b * (NCORES // B) + 1, (b + 1) * (NCORES // B)):
            acc = acc + res.results[c]["outT"].astype(np.float32)
        out[b] = acc.T + out_bias
    return out



# revision 57
# speedup vs baseline: 1.0114x; 1.0114x over previous
"""Multi-head attention (B=2, S=2048, D=1024, H=16) on 8 TRN2 NeuronCores.

Sharding: batch x head-group. Core c handles batch b = c // 4 and heads
[4*(c%4), 4*(c%4)+4). Each core projects Q/K/V for its 4 heads (column-split
wq/wk/wv), runs causal attention per head, and computes its partial of the
output projection (row-split wo). Host sums the 4 partials per batch (the
"all-reduce") and adds wo_b.

Device-side design (v2 — interleaved phases, rebalanced engines):
  - Host supplies q/k/v transposed (xT = x[b].T, [D, S]) so the projection
    contraction dim (D) lands on SBUF partitions with no on-device transpose.
  - Q,K are produced transposed (QT[dout, s]); scores are computed in S^T
    layout [keys, queries]; softmax uses no max-subtraction (scores/8 lie in
    [-3, 3] for randn inputs; exp cannot overflow).
  - Width-65 V strips [64 dims | ones]: the A@V matmul emits both the context
    rows (partitions 0..63) and the softmax denominator (partition 64) per
    head; denominators are inverted on DVE (reciprocal) and broadcast to 128
    partitions with one tiny K=2 matmul — no DRAM round-trips, no Exp<->Ln
    activation-table swaps on the scalar engine.
  - Causal masking by construction: per (query-chunk, key-block), only the
    live query range [128*al, 512) is computed (scores, exp, A@V); just the
    128-wide diagonal transition band needs a triangular mask multiply,
    which runs on the otherwise idle GpSimd engine.
  - Projection (phase A) and attention (phase B) instruction issue is
    interleaved so the tensor engine's projection work overlaps the scalar
    engine's exp work instead of serializing.
  - All big DMAs are split across queues; output partials are bf16.
"""
import math
import os
import numpy as np
from contextlib import ExitStack

B, S, D, H = 2, 2048, 1024, 16
DK = D // H               # 64
NCORES = 8
HPC = H // (NCORES // B)  # heads per core = 4
DHC = HPC * DK            # per-core head dims = 256
P = 128
SCW = 512
NSC = S // SCW            # 4 s-chunks (= query chunks)
NKC = D // P              # 8 contraction chunks
NQB = S // P              # 16 key blocks

_compiled = {}


def _build(mode: str):
    """mode: 'causal' (live-range restricted, const band mask),
             'dense'  (no masking at all),
             'general' (full SxS additive bias streamed from DRAM)."""
    import concourse.bacc as bacc
    import concourse.mybir as mybir
    import concourse.tile as tile

    f32 = mybir.dt.float32
    bf16 = mybir.dt.bfloat16
    fp16 = mybir.dt.float16
    AF = mybir.ActivationFunctionType
    nc = bacc.Bacc("TRN2", target_bir_lowering=False, debug=False,
                   num_devices=NCORES)

    qt = nc.dram_tensor("qt", (NSC, P, NKC, SCW), bf16, kind="ExternalInput").ap()
    kt = nc.dram_tensor("kt", (NSC, P, NKC, SCW), bf16, kind="ExternalInput").ap()
    vt = nc.dram_tensor("vt", (NSC, P, NKC, SCW), bf16, kind="ExternalInput").ap()
    wq = nc.dram_tensor("wq", (P, NKC, DHC), bf16, kind="ExternalInput").ap()
    wk = nc.dram_tensor("wk", (P, NKC, DHC), bf16, kind="ExternalInput").ap()
    wv = nc.dram_tensor("wv", (P, NKC, DHC), bf16, kind="ExternalInput").ap()
    wo = nc.dram_tensor("wo", (P, DHC // P, D), bf16, kind="ExternalInput").ap()
    # aux: [0] = 1.0, [1:257] = wq bias for this core's 256 head-dims
    aux = nc.dram_tensor("aux", (1, 257), bf16, kind="ExternalInput").ap()
    if mode == "causal":
        maskb = nc.dram_tensor("maskb", (P, P), bf16, kind="ExternalInput").ap()
    elif mode == "general":
        maskt = nc.dram_tensor("maskt", (S, S), f32, kind="ExternalInput").ap()
    outT = nc.dram_tensor("outT", (D, S), bf16, kind="ExternalOutput").ap()

    with tile.TileContext(nc) as tc, ExitStack() as ctx:
        consts = ctx.enter_context(tc.tile_pool(name="consts", bufs=1))
        stream = ctx.enter_context(tc.tile_pool(name="stream", bufs=3))
        espool = ctx.enter_context(tc.tile_pool(name="es", bufs=8))
        epool = ctx.enter_context(tc.tile_pool(name="ep", bufs=4))
        opool = ctx.enter_context(tc.tile_pool(name="op", bufs=4))
        rpool = ctx.enter_context(tc.tile_pool(name="rp", bufs=2))
        # PSUM: acc 2x1 bank + av 2x1 + sc 2x2 = 8 banks total
        acc_ps = ctx.enter_context(tc.tile_pool(name="accps", bufs=2, space="PSUM"))
        av_ps = ctx.enter_context(tc.tile_pool(name="avps", bufs=2, space="PSUM"))
        sc_ps = ctx.enter_context(tc.tile_pool(name="scps", bufs=2, space="PSUM"))

        # ---- resident tensors ----
        wq_sb = consts.tile([P, NKC, DHC], bf16, tag="wq")
        wk_sb = consts.tile([P, NKC, DHC], bf16, tag="wk")
        wv_sb = consts.tile([P, NKC, DHC], bf16, tag="wv")
        wo_sb = consts.tile([P, DHC // P, D], bf16, tag="wo")
        aux_sb = consts.tile([1, 257], bf16, tag="aux")
        ones64_sb = consts.tile([P, 64], bf16, tag="ones64")
        qb_sb = consts.tile([P, 2], f32, tag="qb")
        QT_sb = consts.tile([P, 2, S], bf16, tag="QT")
        KT_sb = consts.tile([P, 2, S], bf16, tag="KT")
        # V strips: [key-in-block, sb, pair, [h_even 64|1][h_odd 64|1]]
        V_sb = consts.tile([P, NQB, 2, 130], bf16, tag="V")
        ctx_sb = consts.tile([P, 2, S], bf16, tag="ctx")
        if mode == "causal":
            maskb_sb = consts.tile([P, P], bf16, tag="maskb")

        warm_sb = consts.tile([P, SCW], bf16, tag="warm")

        def init_consts():
            # on-device constants: ones columns of the V strips (softmax
            # denominator rows) and the bcmul broadcast ones — replaces
            # thousands of tiny DMA descriptors with 3 memsets
            nc.gpsimd.memset(V_sb[:, :, :, 64:65], 1.0)
            nc.gpsimd.memset(V_sb[:, :, :, 129:130], 1.0)
            nc.vector.memset(ones64_sb[:], 1.0)
            nc.vector.memset(warm_sb[:], 0.0)

        def warmup():
            # ~4us of dummy matmuls while the first inputs stream in: flips
            # the PE HAM clock gate to 8/8 before the real work arrives
            wp = acc_ps.tile([P, 8, 64], f32, tag="acc", name="accwarm")
            for r in range(10):
                nc.tensor.matmul(wp[:, :, :], warm_sb[:, 0:P], warm_sb[:, :],
                                 start=(r == 0), stop=(r == 9))

        def qb_extract():
            # wq bias column [128, 2] = aux[1:257] via K=1 matmuls
            ps = acc_ps.tile([P, 8, 64], f32, tag="acc")
            for c0 in range(2):
                nc.tensor.matmul(ps[:, c0, 0:1],
                                 aux_sb[:, 1 + P * c0:1 + P * (c0 + 1)],
                                 aux_sb[:, 0:1], start=True, stop=True)
            nc.vector.tensor_copy(qb_sb[:, :], ps[:, 0:2, 0:1])

        def dma_x(sc):
            # two issues per tensor (DMA_DIRECT2D costs ~610ns of issuing-
            # engine time regardless of size): first half arrives early for
            # the projection half-groups, without per-kc issue overhead
            tiles = {}
            for name, src in (("q", qt), ("k", kt), ("v", vt)):
                t = stream.tile([P, NKC, SCW], bf16, tag=f"x{name}")
                nc.sync.dma_start(t[:, 0:4, :], src[sc, :, 0:4, :])
                nc.sync.dma_start(t[:, 4:8, :], src[sc, :, 4:8, :])
                tiles[name] = t
            return tiles

        # ---- Phase A groups: projections for s-chunk sc ----
        def a_groups(sc, xt, split=False):
            gs = []
            ssl = slice(sc * SCW, (sc + 1) * SCW)

            def qk(xkey, w_sb, dst, bias, c0):
                # one whole accumulation chain per closure: the acc psum tile
                # must be allocated, written, and evacuated within one group,
                # or interleaved groups sharing the pool wrap it mid-chain
                def g():
                    ps = acc_ps.tile([P, 8, 64], f32, tag="acc")
                    x = xt[xkey]
                    for kc in range(NKC):
                        nc.tensor.matmul(ps[:, :, :],
                                         w_sb[:, kc, c0 * P:(c0 + 1) * P],
                                         x[:, kc, :],
                                         start=(kc == 0),
                                         stop=(kc == NKC - 1))
                    if bias:
                        # q bias fused into the evacuation copy (per-partition
                        # scalar add). (k bias dropped: per-query score shift,
                        # softmax-invariant; v bias added host-side via wo@bv)
                        nc.vector.tensor_scalar_add(dst[:, c0, ssl], ps[:, :, :],
                                                    qb_sb[:, c0:c0 + 1])
                    else:
                        nc.vector.tensor_copy(dst[:, c0, ssl], ps[:, :, :])
                return g

            for c0 in range(2):
                gs.append(qk("q", wq_sb, QT_sb, True, c0))
            for c0 in range(2):
                gs.append(qk("k", wk_sb, KT_sb, False, c0))

            def vproj(j):
                def g():
                    sb = 4 * sc + j
                    ps = acc_ps.tile([P, 8, 64], f32, tag="acc")
                    pv = ps[:, 0:4, :]
                    for kc in range(NKC):
                        nc.tensor.matmul(pv, xt["v"][:, kc, j * P:(j + 1) * P],
                                         wv_sb[:, kc, :],
                                         start=(kc == 0), stop=(kc == NKC - 1))
                    # wv cols are host-permuted [h0,h2,h1,h3] -> 2 strided copies
                    nc.vector.tensor_copy(V_sb[:, sb, :, 0:DK], ps[:, 0:2, :])
                    nc.vector.tensor_copy(V_sb[:, sb, :, 65:65 + DK], ps[:, 2:4, :])
                return g

            vg = [vproj(j) for j in range(SCW // P)]
            if split:
                return gs, vg
            return gs + vg

        # ---- Phase B groups: attention for query chunk qc ----
        if mode == "general":
            mkpool = ctx.enter_context(tc.tile_pool(name="mk", bufs=1))

        def early_groups(qc, pairs_kbs, store):
            # score+exp only, for off-diagonal key blocks of a later query
            # chunk: pulled forward into the previous (PE-bound) segment so
            # the final segment's scalar-engine exp backlog shrinks
            gs = []
            for pair, kb in pairs_kbs:
                def g(pair=pair, kb=kb):
                    sct = sc_ps.tile([P, 2, SCW], f32, tag="sc")
                    for par in range(2):
                        hp = 64 * par
                        nc.tensor.matmul(
                            sct[:, par, :],
                            KT_sb[hp:hp + 64, pair, kb * P:(kb + 1) * P],
                            QT_sb[hp:hp + 64, pair, qc * SCW:(qc + 1) * SCW],
                            start=True, stop=True, tile_position=(hp, 0))
                    es = epool.tile([P, 2, SCW], bf16, tag="es_e",
                                    name=f"ese{pair}_{kb}")
                    nc.scalar.activation(es[:, :, :], sct[:, :, :], AF.Exp,
                                         scale=1.0 / math.sqrt(DK))
                    store[(pair, kb)] = es
                gs.append(g)
            return gs

        def b_groups(qc, pre=None, early_gs=()):
            gs = []
            pre = pre or {}
            qsl = slice(qc * SCW, (qc + 1) * SCW)
            nkb = 4 * (qc + 1) if mode == "causal" else NQB
            mk_tiles = {}
            if mode == "general":
                def mk_load(g_):
                    def g():
                        mt = mkpool.tile([P, 2, 512], f32, tag=f"mk{g_}")
                        nc.sync.dma_start(
                            mt[:], maskt[2 * g_ * P:(2 * g_ + 2) * P, qsl]
                            .rearrange("(u p) q -> p u q", p=P))
                        mk_tiles[g_] = mt
                    return g
                for g_ in range(nkb // 2):
                    gs.append(("mk", mk_load(g_)))

            avs_by_pair = {}
            # denominator rows at partitions {0, 64} (quadrant-aligned bases;
            # rows 1..63 are junk, never read); free dims: [ch, q]
            Rstg = rpool.tile([65, 2, SCW], f32, tag="Rstg")

            es_by_kb = {}

            def lo_of(kb):
                al = kb - 4 * qc
                return P * al if (mode == "causal" and al > 0) else 0

            def score_part(pair, kb):
                lo = lo_of(kb)
                al = kb - 4 * qc
                sct = sc_ps.tile([P, 2, SCW], f32, tag="sc")
                for par in range(2):
                    hp = 64 * par
                    nc.tensor.matmul(sct[:, par, lo:],
                                     KT_sb[hp:hp + 64, pair, kb * P:(kb + 1) * P],
                                     QT_sb[hp:hp + 64, pair, qc * SCW + lo:(qc + 1) * SCW],
                                     start=True, stop=True,
                                     tile_position=(hp, 0))
                if mode == "general":
                    nc.vector.tensor_add(sct[:, 0, :], sct[:, 0, :],
                                         mk_tiles[kb // 2][:, kb % 2, :])
                    nc.vector.tensor_add(sct[:, 1, :], sct[:, 1, :],
                                         mk_tiles[kb // 2][:, kb % 2, :])
                es = espool.tile([P, 2, SCW], bf16, tag="es")
                nc.scalar.activation(es[:, :, lo:], sct[:, :, lo:], AF.Exp,
                                     scale=1.0 / math.sqrt(DK))
                if mode == "causal" and 0 <= al:
                    # triangular band mask on the diagonal 128 columns
                    nc.gpsimd.tensor_mul(es[:, 0, lo:lo + P], es[:, 0, lo:lo + P],
                                         maskb_sb[:, :])
                    nc.gpsimd.tensor_mul(es[:, 1, lo:lo + P], es[:, 1, lo:lo + P],
                                         maskb_sb[:, :])
                es_by_kb[(pair, kb)] = es

            def av_part(pair, kb):
                lo = lo_of(kb)
                es = es_by_kb.pop((pair, kb))
                if kb == 0:
                    avs_by_pair[pair] = [
                        av_ps.tile([P, SCW], f32, tag="av", name=f"av{pair}{par}")
                        for par in range(2)]
                avs = avs_by_pair[pair]
                for par in range(2):
                    nc.tensor.matmul(avs[par][0:65, lo:],
                                     V_sb[:, kb, pair, par * 65:par * 65 + 65],
                                     es[:, par, lo:],
                                     start=(kb == 0), stop=(kb == nkb - 1))

            def attn(pair, kb):
                # software pipeline: issue av two key-blocks behind the
                # scores so the PE never waits on the exp in program order
                def g():
                    if (pair, kb) not in pre:
                        score_part(pair, kb)
                    if kb >= 2:
                        av_part(pair, kb - 2)
                    if kb == nkb - 1:
                        av_part(pair, nkb - 2)
                        av_part(pair, nkb - 1)
                return g

            def evac_den(pair):
                # den rows first, so recip (DVE) runs before the ctx casts
                # and the bc matmul unblocks ~2us earlier at pair ends
                def g():
                    avs = avs_by_pair[pair]
                    for par in range(2):
                        nc.vector.tensor_copy(Rstg[64 * par:64 * par + 1, pair, :],
                                              avs[par][64:65, :])
                return g

            def evac_ctx(pair):
                def g():
                    avs = avs_by_pair[pair]
                    for par in range(2):
                        if qc == NSC - 1 and pair == 1 and par == 0:
                            # endgame: exp is done, scalar engine is free
                            nc.scalar.copy(
                                ctx_sb[0:64, pair, qsl], avs[par][0:64, :])
                        else:
                            nc.vector.tensor_copy(
                                ctx_sb[64 * par:64 * par + 64, pair, qsl],
                                avs[par][0:64, :])
                return g

            Rf = rpool.tile([65, 2, SCW], f32, tag="Rf")
            Rb = rpool.tile([65, 2, SCW], bf16, tag="Rb")

            def recip(pair):
                def g():
                    with nc.allow_low_precision("softmax denom scale in bf16"):
                        nc.vector.reciprocal_approx_fast(Rf[:, pair, :],
                                                         Rstg[:, pair, :])
                        nc.vector.tensor_copy(Rb[:, pair, :], Rf[:, pair, :])
                return g

            def bcmul(pair):
                def g():
                    bc = acc_ps.tile([P, 8, 64], f32, tag="acc")
                    for par in range(2):
                        nc.tensor.matmul(bc[64 * par:64 * par + 64, :, :],
                                         ones64_sb[64 * par:64 * par + 1, :],
                                         Rb[64 * par:64 * par + 1, pair, :],
                                         start=True, stop=True,
                                         tile_position=(64 * par, 64 * par))
                    nc.vector.tensor_mul(ctx_sb[:, pair, qsl], ctx_sb[:, pair, qsl],
                                         bc[:, :, :])
                return g

            es_by_kb.update(pre)
            ei = 0
            for pair in range(2):
                dkb = 6 if qc == NSC - 1 else 2
                for kb in range(nkb):
                    gs.append(("attn", attn(pair, kb)))
                    if pair == 1 and kb == dkb:
                        gs.append(("bc", bcmul(0)))
                    if pair == 1 and kb >= 3 and kb % 2 == 1 and ei < len(early_gs):
                        gs.append(("early", early_gs[ei]))
                        ei += 1
                gs.append(("evac", evac_den(pair)))
                gs.append(("recip", recip(pair)))
                gs.append(("evacc", evac_ctx(pair)))
            gs.append(("bc", bcmul(1)))
            while ei < len(early_gs):
                gs.append(("early", early_gs[ei]))
                ei += 1

            def outproj(nb):
                def g():
                    ps = acc_ps.tile([P, 8, 64], f32, tag="acc")
                    for hc in range(2):
                        nc.tensor.matmul(ps[:, :, :],
                                         wo_sb[:, hc, nb * P:(nb + 1) * P],
                                         ctx_sb[:, hc, qsl],
                                         start=(hc == 0), stop=(hc == 1))
                    if nb % 2 == 0:
                        oth["t"] = opool.tile([P, 2, SCW], bf16, tag="ot",
                                              name=f"ot{qc}_{nb}")
                    ot = oth["t"]
                    if qc == NSC - 1 and nb % 2 == 1:
                        # endgame: split the evacuation casts across engines
                        # (exp is done, the scalar engine is idle)
                        nc.scalar.copy(ot[:, 1, :], ps[:, :, :])
                    else:
                        nc.vector.tensor_copy(ot[:, nb % 2, :], ps[:, :, :])
                    if nb % 2 == 1:
                        # paired DMA: two nb blocks per issue, 1KB lines
                        h0 = qc * SCW
                        dst = outT[(nb - 1) * P:(nb + 1) * P, h0:h0 + SCW]
                        nc.sync.dma_start(dst.rearrange("(j p) c -> p j c", p=P),
                                          ot[:, :, :])
                return g

            oth = {}
            ops = [outproj(nb) for nb in range(D // P)]
            return gs, ops

        def interleave(bs, as_):
            """Merge phase-A half-groups into the tagged phase-B stream at an
            even rate across all slots: the PE stream is strictly in-order, so
            ~1us of projection work after every attention slot papers over the
            score->exp->av dependency stalls."""
            if not as_:
                for _, g in bs:
                    g()
                return
            slots = ("attn", "evac", "recip", "evacc", "bc", "early", "opd")
            nslots = sum(1 for t, _ in bs if t in slots)
            rate = len(as_) / max(1, nslots)
            ai = 0
            acc = 0.0
            for tag, g in bs:
                g()
                if tag in slots:
                    acc += rate
                    while ai < len(as_) and acc >= 1.0:
                        as_[ai]()
                        ai += 1
                        acc -= 1.0
            while ai < len(as_):
                as_[ai]()
                ai += 1

        # ---- issue ----
        xt = {}
        for name, src in (("q", qt), ("k", kt), ("v", vt)):
            t = stream.tile([P, NKC, SCW], bf16, tag=f"x{name}")
            xt[name] = t
        # Head DMA: DMA_DIRECT2D issue costs ~610ns on the issuing engine,
        # so (a) few, large transfers, (b) split the issue load across both
        # HWDGE queues — sync drives the q-side critical path while the
        # scalar engine (idle until the first exp) drives k/v/weights.
        # single queue, criticality-ordered: arrival order then matches
        # compute order (q -> k -> v); only the late-needed maskb/wo go on
        # the scalar queue
        # interleave wq slices with q chunks in the order the first
        # projection chain consumes them (kc ascending), so it streams
        # without stalling on late weights
        nc.sync.dma_start(wq_sb[:, 0:2, :], wq[:, 0:2, :])
        nc.sync.dma_start(aux_sb[:], aux)
        for g in range(4):
            nc.sync.dma_start(xt["q"][:, 2 * g, :], qt[0, :, 2 * g, :])
            nc.sync.dma_start(xt["q"][:, 2 * g + 1, :], qt[0, :, 2 * g + 1, :])
            if g < 3:
                nc.sync.dma_start(wq_sb[:, 2 * g + 2:2 * g + 4, :],
                                  wq[:, 2 * g + 2:2 * g + 4, :])
        nc.sync.dma_start(wk_sb[:, :, :], wk[:, :, :])
        for g in range(4):
            nc.sync.dma_start(xt["k"][:, 2 * g:2 * g + 2, :],
                              kt[0, :, 2 * g:2 * g + 2, :])
        nc.sync.dma_start(wv_sb[:, :, :], wv[:, :, :])
        for g in range(4):
            nc.sync.dma_start(xt["v"][:, 2 * g:2 * g + 2, :],
                              vt[0, :, 2 * g:2 * g + 2, :])
        if mode == "causal":
            nc.scalar.dma_start(maskb_sb[:], maskb)
        nc.scalar.dma_start(wo_sb[:, :, :], wo[:, :, :])
        init_consts()
        warmup()
        qb_extract()
        for g in a_groups(0, xt):
            g()
        vg0 = []

        def inject_ops(bs, ops):
            # spread deferred out-proj groups after the 3rd..10th attn group
            merged = []
            k = 0
            oi = 0
            for tag, g in bs:
                merged.append((tag, g))
                if tag == "attn":
                    k += 1
                    if k >= 3 and oi < len(ops):
                        merged.append(("opd", ops[oi]))
                        oi += 1
            while oi < len(ops):
                merged.append(("opd", ops[oi]))
                oi += 1
            return merged

        pend = []
        pre3 = {}
        carry = vg0
        for sc in range(1, NSC):
            xt = dma_x(sc)
            egs = ()
            if mode == "causal" and sc == NSC - 1:
                egs = early_groups(NSC - 1, [(0, kb) for kb in range(4)], pre3)
            bs, ops = b_groups(sc - 1, early_gs=egs)
            interleave(inject_ops(bs, pend), carry + a_groups(sc, xt))
            carry = []
            pend = ops
        bs, ops = b_groups(NSC - 1, pre=pre3)
        for _, g in inject_ops(bs, pend):
            g()
        for g in ops:
            g()

    nc.compile()
    return nc


def _get_compiled(mode: str):
    if mode not in _compiled:
        _compiled[mode] = _build(mode)
    return _compiled[mode]


def _detect_mode(mask: np.ndarray) -> str:
    m = np.asarray(mask).reshape(S, S)
    if np.array_equal(m != 0, np.tril(np.ones((S, S), dtype=bool))):
        return "causal"
    if np.all(m != 0):
        return "dense"
    return "general"


def kernel(q, k, v, mask, wq_w, wq_b, wk_w, wk_b, wv_w, wv_b, wo_w, wo_b):
    from concourse import bass_utils

    import ml_dtypes

    q = np.asarray(q, dtype=np.float32)
    k = np.asarray(k, dtype=np.float32)
    v = np.asarray(v, dtype=np.float32)
    mode = _detect_mode(np.asarray(mask))
    nc = _get_compiled(mode)

    def tile_in(x):  # [S, D] -> [sc, p, kc, scw] (x^T pre-tiled for DMA)
        return np.ascontiguousarray(
            x.reshape(S // SCW, SCW, D // P, P).transpose(0, 3, 2, 1)
        ).astype(ml_dtypes.bfloat16)

    def tile_w(w, hs, perm=None):  # [Dout, Din] slice -> W^T tiled [p, kc, DHC]
        ws = w[hs, :]
        if perm is not None:
            ws = ws[perm]
        return np.ascontiguousarray(
            ws.T.reshape(D // P, P, DHC).transpose(1, 0, 2)
        ).astype(ml_dtypes.bfloat16)

    qT = [tile_in(q[b]) for b in range(B)]
    kT = [tile_in(k[b]) for b in range(B)]
    vT = [tile_in(v[b]) for b in range(B)]

    perm = np.r_[0:64, 128:192, 64:128, 192:256]  # head order h0,h2,h1,h3

    if mode == "causal":
        i = np.arange(P)[:, None]
        jb = np.arange(P)[None, :]
        maskb = (jb >= i).astype(ml_dtypes.bfloat16)
    elif mode == "general":
        m = np.asarray(mask).reshape(S, S)
        maskt = np.where(m.T == 0, np.float32(-1.0e9), np.float32(0.0))

    in_maps = []
    for c in range(NCORES):
        b = c // (NCORES // B)
        hg = c % (NCORES // B)
        hs = slice(hg * DHC, (hg + 1) * DHC)
        aux_arr = np.zeros((1, 257), ml_dtypes.bfloat16)
        aux_arr[0, 0] = 1.0
        aux_arr[0, 1:1 + DHC] = wq_b[hs].astype(ml_dtypes.bfloat16)
        m_ = {
            "qt": qT[b], "kt": kT[b], "vt": vT[b],
            "wq": tile_w(wq_w, hs),
            "wk": tile_w(wk_w, hs),
            "wv": tile_w(wv_w, hs, perm),
            "wo": np.ascontiguousarray(
                wo_w[:, hs].T.reshape(2, P, D).transpose(1, 0, 2)
            ).astype(ml_dtypes.bfloat16),
            "aux": aux_arr,
        }
        if mode == "causal":
            m_["maskb"] = maskb
        elif mode == "general":
            m_["maskt"] = maskt
        in_maps.append(m_)

    trace = os.environ.get("KERNEL_TRACE", "") == "1"
    res = bass_utils.run_bass_kernel_spmd(nc, in_maps, core_ids=list(range(NCORES)),
                                          trace=trace)
    if trace:
        kernel.last_exec_time_ns = res.exec_time_ns
        kernel.last_results = res

    # v-projection bias folded here: softmax weights sum to 1, so each
    # head's bv adds a constant; through wo it is wo_w @ wv_b
    out_bias = wo_b + wo_w.astype(np.float64) @ wv_b.astype(np.float64)
    out = np.empty((B, S, D), np.float32)
    for b in range(B):
        acc = res.results[b * (NCORES // B)]["outT"].astype(np.float32)
        for c in range(b * (NCORES // B) + 1, (b + 1) * (NCORES // B)):
            acc = acc + res.results[c]["outT"].astype(np.float32)
        out[b] = acc.T + out_bias
    return out

